# revision 12
# baseline (speedup 1.0000x reference)
"""Multi-head self-attention (B=4, N=2048, C=1024, H=16) on 8 trn2 cores.

Sharding: core c -> (batch b = c // 2, head-group g = c % 2).
Each core computes, for its batch and its 8 heads (512 of the 1024 channels):
    Q/K/V projections, softmax attention, and a partial output projection
    through its 512 rows of Wo.  The final y = sum of 4 fp16 partials
    (2 cores x 2 jt-pair partials) + bo, summed on the host.

v4.3 schedule:
  - Inputs shipped pre-cast fp16 and pre-packed so every weight/x tensor
    loads with one DMA of 8..32 KB contiguous per-partition lines.
  - Prelude: V projection, then pair-0 Q/K projection.
  - Attention, software-pipelined two iterations deep (score matmuls of
    iteration k issue before ctx/ones matmuls of k-2), hiding the exp
    latency entirely; the PE runs back-to-back.
  - exp split across engines: even key-tiles on the Scalar engine (exact
    Exp); odd key-tiles on the Vector engine via a Schraudolph fp16
    bit-trick (i16 = score*1024*log2(e)/8 + 15301, bitcast fp16), a
    zero-mean +-3% approximation; overall output error ~1.2e-2 < 2e-2.
  - Q/K projections for pairs 1..3, then the jt{0,1} half of the output
    projection, drip into the attention PE stream (2 matmuls/iter) to
    fill the PE slack; their bias/copy moves go to the Scalar engine.
  - Softmax denominators via ones-matmuls (concurrent with ctx through
    PE column tiling); normalization via reciprocal_approx_fast.
  - Tail: only the jt{2,3} half of the output projection remains.
"""

import numpy as np

B, N, C, H = 4, 2048, 1024, 16
D = C // H            # 64
G = 2                 # head-groups (tensor-parallel factor)
J = C // G            # 512 local channels
HL = H // G           # 8 local heads
CT = C // 128         # 8 c-tiles
JT = J // 128         # 4 local j-tiles (= head pairs)
NT = N // 128         # 16 token tiles
KT = N // 128         # 16 key tiles
QC = 512              # q-chunk width
NQC = N // QC         # 4 q-chunks
PW = 512              # projection chunk width
NPC = N // PW         # 4 projection chunks per (jt, q|k)
N_CORES = 8

# Schraudolph fp16 exp: i16 = round(score * A + B); bitcast -> fp16 ~ exp(score/8)
SCH_A = 1024.0 * 0.125 * 1.4426950408889634
SCH_B = 15360.0 - 59.0
DVE_EXP = True        # odd key-tiles use the DVE bit-trick exp

_CACHE = {}


def _build():
    import sys
    if "/opt/trn_rl_repo" not in sys.path:
        sys.path.insert(0, "/opt/trn_rl_repo")
    from contextlib import ExitStack
    import concourse.bacc as bacc
    import concourse.tile as tile
    from concourse import mybir

    f32 = mybir.dt.float32
    f16 = mybir.dt.float16
    i16 = mybir.dt.int16
    Exp = mybir.ActivationFunctionType.Exp
    mult = mybir.AluOpType.mult
    add = mybir.AluOpType.add

    nc = bacc.Bacc("TRN2", target_bir_lowering=False, debug=False)

    # pre-packed inputs: partition dim first, whole tensor in one DMA
    x_d = nc.dram_tensor("xp", [128, CT, N], f16, kind="ExternalInput")
    wq_d = nc.dram_tensor("wqp", [128, CT, J], f16, kind="ExternalInput")
    wk_d = nc.dram_tensor("wkp", [128, CT, J], f16, kind="ExternalInput")
    wv_d = nc.dram_tensor("wvp", [128, CT, J], f16, kind="ExternalInput")
    wo_d = nc.dram_tensor("wop", [128, JT, C], f16, kind="ExternalInput")
    bq_d = nc.dram_tensor("bq", [J], f32, kind="ExternalInput")
    bk_d = nc.dram_tensor("bk", [J], f32, kind="ExternalInput")
    bv_d = nc.dram_tensor("bv", [J], f32, kind="ExternalInput")
    y0_d = nc.dram_tensor("y0", [N, C], f16, kind="ExternalOutput")
    y1_d = nc.dram_tensor("y1", [N, C], f16, kind="ExternalOutput")

    with tile.TileContext(nc) as tc, ExitStack() as top:
        consts = top.enter_context(tc.tile_pool(name="consts", bufs=1))
        persist = top.enter_context(tc.tile_pool(name="persist", bufs=1))
        etp = top.enter_context(tc.tile_pool(name="etp", bufs=4))
        rrp = top.enter_context(tc.tile_pool(name="rrp", bufs=2))
        ysb = top.enter_context(tc.tile_pool(name="ysb", bufs=4))

        qt_t = persist.tile([128, JT, N], f16, tag="qt")
        kt_t = persist.tile([128, JT, N], f16, tag="kt")
        v_t = persist.tile([128, NT, J], f16, tag="v")
        ctxT_t = persist.tile([128, JT, N], f16, tag="ctxT")
        x_t = persist.tile([128, CT, N], f16, tag="x")
        wq_t = persist.tile([128, CT, J], f16, tag="wq")
        wk_t = persist.tile([128, CT, J], f16, tag="wk")
        wv_t = persist.tile([128, CT, J], f16, tag="wv")
        wo_t = persist.tile([128, JT, C], f16, tag="wo")

        ones_t = consts.tile([128, 64], f16, tag="ones")
        nc.vector.memset(ones_t[:], 1.0)
        bq_t = consts.tile([128, JT], f32, tag="bq")
        bk_t = consts.tile([128, JT], f32, tag="bk")
        bv_t = consts.tile([128, J], f32, tag="bv")

        # ---- input DMAs: wv + bv + x first (V projection starts soonest) ----
        nc.sync.dma_start(out=wv_t[:], in_=wv_d.ap())
        nc.sync.dma_start(
            out=bv_t[:], in_=bv_d.ap().unsqueeze(0).partition_broadcast(128).squeeze(1)
        )
        nc.sync.dma_start(out=x_t[:], in_=x_d.ap())
        nc.sync.dma_start(out=bq_t[:], in_=bq_d.ap().rearrange("(t p) -> p t", p=128))
        nc.sync.dma_start(out=bk_t[:], in_=bk_d.ap().rearrange("(t p) -> p t", p=128))
        nc.sync.dma_start(out=wq_t[:], in_=wq_d.ap())
        nc.sync.dma_start(out=wk_t[:], in_=wk_d.ap())
        nc.sync.dma_start(out=wo_t[:], in_=wo_d.ap())

        # ---- prelude: V projection + Q/K for pair 0 ----
        with tc.tile_pool(name="pps", bufs=2, space="PSUM") as pps:
            for nt in range(NT):
                v_ps = pps.tile([128, J], f32, tag="p", name="v_ps")
                for ct in range(CT):
                    nc.tensor.matmul(
                        v_ps[:], x_t[:, ct, nt * 128:(nt + 1) * 128],
                        wv_t[:, ct, :], start=(ct == 0), stop=(ct == CT - 1),
                    )
                nc.vector.tensor_tensor(v_t[:, nt, :], v_ps[:], bv_t[:], add)
            for w_t, b_t, o_t in ((wq_t, bq_t, qt_t), (wk_t, bk_t, kt_t)):
                for h in range(NPC):
                    ns = h * PW
                    q_ps = pps.tile([128, PW], f32, tag="p", name="q_ps")
                    for ct in range(CT):
                        nc.tensor.matmul(
                            q_ps[:], w_t[:, ct, 0:128],
                            x_t[:, ct, ns:ns + PW],
                            start=(ct == 0), stop=(ct == CT - 1),
                        )
                    nc.vector.tensor_scalar_add(
                        o_t[:, 0, ns:ns + PW], q_ps[:], b_t[:, 0:1]
                    )

        # ---- filler streams dripped into the attention PE stream ----
        # Phase 1: Q/K projection chunks for pairs 1..3 (8 matmuls each).
        # Phase 2 (gated on pair >= 2): out-projection partial jt{0,1}
        # (2 matmuls + copy + DMA per (nt, cc) group).
        fill = {"proj": [], "tail": [], "cur": None, "pool": None, "ncopy": 0}
        for jt in range(1, JT):
            for w_t, b_t, o_t in ((wq_t, bq_t, qt_t), (wk_t, bk_t, kt_t)):
                for h in range(NPC):
                    fill["proj"].append((jt, w_t, b_t, o_t, h))
        fill["proj"].reverse()
        for nt in range(NT):
            for cc in range(2):
                fill["tail"].append((nt, cc))
        fill["tail"].reverse()

        def emit_y_group(nt, cc, y_ps, ydst):
            """Finish one out-proj partial group: copy + DMA."""
            y_sb = ysb.tile([128, 512], f16, tag="ysb")
            if fill["ncopy"] % 2 == 0:
                nc.scalar.copy(y_sb[:], y_ps[:])
            else:
                nc.vector.tensor_copy(y_sb[:], y_ps[:])
            fill["ncopy"] += 1
            nc.sync.dma_start(
                out=ydst.ap()[nt * 128:(nt + 1) * 128,
                              cc * 512:(cc + 1) * 512],
                in_=y_sb[:],
            )

        def emit_fill_mms(n, allow_tail):
            st = fill
            while n > 0:
                if st["cur"] is None:
                    if st["proj"]:
                        item = st["proj"].pop()
                        st["cur"] = ("proj", item, 0)
                        st["psum"] = st["pool"].tile(
                            [128, PW], f32, tag="qk", name="fill_ps"
                        )
                    elif st["tail"] and allow_tail:
                        item = st["tail"].pop()
                        st["cur"] = ("tail", item, 0)
                        st["psum"] = st["pool"].tile(
                            [128, PW], f32, tag="qk", name="fill_ps"
                        )
                    else:
                        return
                kind, item, i = st["cur"]
                if kind == "proj":
                    jt, w_t, b_t, o_t, h = item
                    ns = h * PW
                    nc.tensor.matmul(
                        st["psum"][:], w_t[:, i, jt * 128:(jt + 1) * 128],
                        x_t[:, i, ns:ns + PW],
                        start=(i == 0), stop=(i == CT - 1),
                    )
                    n -= 1
                    if i + 1 == CT:
                        nc.scalar.add(
                            o_t[:, jt, ns:ns + PW], st["psum"][:],
                            b_t[:, jt:jt + 1]
                        )
                        st["cur"] = None
                    else:
                        st["cur"] = (kind, item, i + 1)
                else:
                    nt, cc = item
                    nc.tensor.matmul(
                        st["psum"][:],
                        ctxT_t[:, i, nt * 128:(nt + 1) * 128],
                        wo_t[:, i, cc * 512:(cc + 1) * 512],
                        start=(i == 0), stop=(i == 1),
                    )
                    n -= 1
                    if i + 1 == 2:
                        emit_y_group(nt, cc, st["psum"], y0_d)
                        st["cur"] = None
                    else:
                        st["cur"] = (kind, item, i + 1)

        # ---- attention: 2-iteration software pipeline ----
        with (
            tc.tile_pool(name="stp", bufs=2, space="PSUM") as stp,
            tc.tile_pool(name="cxp", bufs=2, space="PSUM") as cxp,
            tc.tile_pool(name="ssp", bufs=1, space="PSUM") as ssp,
            tc.tile_pool(name="qkp", bufs=1, space="PSUM") as qkp,
        ):
            fill["pool"] = qkp

            def emit_scores(p, qc, k):
                qs = qc * QC
                st_ps = stp.tile([128, 2, QC], f32, tag="st")
                nc.tensor.matmul(
                    st_ps[:, 0, :],
                    kt_t[0:64, p, k * 128:(k + 1) * 128],
                    qt_t[0:64, p, qs:qs + QC],
                    start=True, stop=True,
                )
                nc.tensor.matmul(
                    st_ps[:, 1, :],
                    kt_t[64:128, p, k * 128:(k + 1) * 128],
                    qt_t[64:128, p, qs:qs + QC],
                    start=True, stop=True,
                )
                et_t = etp.tile([128, 2, QC], f16, tag="et")
                return st_ps, et_t

            def emit_exp(k, st_ps, et_t):
                if DVE_EXP and (k % 2 == 1):
                    nc.vector.tensor_scalar(
                        out=et_t[:].bitcast(i16), in0=st_ps[:],
                        scalar1=SCH_A, scalar2=SCH_B, op0=mult, op1=add,
                    )
                else:
                    nc.scalar.activation(et_t[:], st_ps[:], Exp, scale=0.125)

            def emit_ctx(p, qc, k, et_t, ctx_ps, s_ps):
                hA, hB = 2 * p, 2 * p + 1
                first, last = (k == 0), (k == KT - 1)
                nc.tensor.matmul(
                    ctx_ps[0:64, :], v_t[:, k, hA * 64:(hA + 1) * 64],
                    et_t[:, 0, :], start=first, stop=last,
                    tile_position=(0, 0),
                )
                nc.tensor.matmul(
                    ctx_ps[64:128, :], v_t[:, k, hB * 64:(hB + 1) * 64],
                    et_t[:, 1, :], start=first, stop=last,
                    tile_position=(0, 64),
                )
                nc.tensor.matmul(
                    s_ps[0:64, :], ones_t[:],
                    et_t[:, 0, :], start=first, stop=last,
                    tile_position=(0, 0),
                )
                nc.tensor.matmul(
                    s_ps[64:128, :], ones_t[:],
                    et_t[:, 1, :], start=first, stop=last,
                    tile_position=(0, 64),
                )

            def emit_normalize(p, qc, ctx_ps, s_ps):
                qs = qc * QC
                rr_t = rrp.tile([128, QC], f32, tag="rr")
                nc.vector.reciprocal_approx_fast(out=rr_t[:], in_=s_ps[:])
                nc.vector.tensor_tensor(
                    ctxT_t[:, p, qs:qs + QC], ctx_ps[:], rr_t[:], mult
                )

            from collections import deque

            pend = deque()  # (et_t, p, qc, k, ctx_ps, s_ps), ctx lags 2 iters

            def drain_one():
                et_t, pp, pqc, pk, pct, pss = pend.popleft()
                emit_ctx(pp, pqc, pk, et_t, pct, pss)
                if pk == KT - 1:
                    emit_normalize(pp, pqc, pct, pss)

            for p in range(JT):
                for qc in range(NQC):
                    ctx_ps = cxp.tile([128, QC], f32, tag="ctx")
                    s_ps = ssp.tile([128, QC], f32, tag="s")
                    for k in range(KT):
                        st_ps, et_t = emit_scores(p, qc, k)
                        emit_exp(k, st_ps, et_t)
                        pend.append((et_t, p, qc, k, ctx_ps, s_ps))
                        if len(pend) > 2:
                            drain_one()
                        emit_fill_mms(2, allow_tail=((p, qc) >= (2, 1)))
            while pend:
                drain_one()
            emit_fill_mms(1 << 30, allow_tail=True)

        # ---- output projection: remaining jt{2,3} partial ----
        with tc.tile_pool(name="yps", bufs=2, space="PSUM") as yps:
            for nt in range(NT):
                for cc in range(2):
                    y_ps = yps.tile([128, 512], f32, tag="y")
                    for jt in (2, 3):
                        nc.tensor.matmul(
                            y_ps[:],
                            ctxT_t[:, jt, nt * 128:(nt + 1) * 128],
                            wo_t[:, jt, cc * 512:(cc + 1) * 512],
                            start=(jt == 2), stop=(jt == 3),
                        )
                    emit_y_group(nt, cc, y_ps, y1_d)

    nc.compile()
    return nc


def _get_module():
    if "nc" not in _CACHE:
        _CACHE["nc"] = _build()
    return _CACHE["nc"]


def _pack(w):
    # [CT*128, M] f32 -> [128, CT, M] contiguous fp16
    m = w.shape[1]
    return np.ascontiguousarray(
        w.astype(np.float16).reshape(-1, 128, m).transpose(1, 0, 2)
    )


def make_in_maps(x, Wq, bq, Wk, bk, Wv, bv, Wo):
    x = np.asarray(x, dtype=np.float32)
    Wq, Wk, Wv, Wo = (np.asarray(a, dtype=np.float32) for a in (Wq, Wk, Wv, Wo))
    in_maps = []
    for c in range(N_CORES):
        b, g = divmod(c, 2)
        js = slice(g * J, (g + 1) * J)
        in_maps.append({
            "xp": _pack(np.ascontiguousarray(x[b].T)),
            "wqp": _pack(Wq[:, js]),
            "wkp": _pack(Wk[:, js]),
            "wvp": _pack(Wv[:, js]),
            "wop": _pack(Wo[js, :]),
            "bq": np.ascontiguousarray(np.asarray(bq, dtype=np.float32)[js]),
            "bk": np.ascontiguousarray(np.asarray(bk, dtype=np.float32)[js]),
            "bv": np.ascontiguousarray(np.asarray(bv, dtype=np.float32)[js]),
        })
    return in_maps


def kernel(x, Wq, bq, Wk, bk, Wv, bv, Wo, bo, **_unused):
    import sys
    if "/opt/trn_rl_repo" not in sys.path:
        sys.path.insert(0, "/opt/trn_rl_repo")
    from concourse.bass_utils import run_bass_kernel_spmd

    nc = _get_module()
    in_maps = make_in_maps(x, Wq, bq, Wk, bk, Wv, bv, Wo)
    res = run_bass_kernel_spmd(nc, in_maps, list(range(N_CORES)))
    bo = np.asarray(bo, dtype=np.float32)
    out = np.empty((B, N, C), dtype=np.float32)
    for b in range(B):
        r0, r1 = res.results[2 * b], res.results[2 * b + 1]
        out[b] = (
            r0["y0"].astype(np.float32) + r0["y1"].astype(np.float32)
            + r1["y0"].astype(np.float32) + r1["y1"].astype(np.float32) + bo
        )
    return out
